# revision 64
# baseline (speedup 1.0000x reference)
"""Bass/Trainium2 kernel for nn_LogRatio loss, data-parallel over anchor rows on 8 cores.

Math: loss = sum_{m,j,k,l} pos[j,k] * N_m[j,l] * (A[j,k] - A[j,l] - c_m)^2
with A = log(X @ X.T + eps). All masks depend on labels only through the
anchor's class t_j (24 classes), so every masked row-reduction becomes a
matmul of A against per-class 0/1 tables W[l, c] followed by a per-row
selection of column c = t_j.

Key layout choices vs the naive version:
- The loss only involves differences A[j,k] - A[j,l], so A may be shifted by
  any constant: fold exp(-s/2) into X on the host (s ~ mean log-sim). The
  centered A' in [-0.6, 0.4] makes bf16 rounding of the matmul operands
  harmless, enabling all-bf16 PE work (fp32 matmuls are 4x slower per column
  and need 4x-slower weight loads).
- W tables are the stationary matmul operand (72/48 columns), so the G sums
  accumulate transposed ([class, j]) in PSUM across the 16 l-chunks, with
  only 120 ldweights columns per chunk instead of 4 full 128x128 fp32 loads.
- Per-core inputs are rotated along l so each core's own anchor block is the
  first 256 columns of its X^T copy: the j-block ("xjt") needs no separate
  DMA and compute starts as soon as the first slice lands.
- Medium streamed DMAs: the naive version's DMA phase was sync-engine
  descriptor-issue bound (~2.6k descriptors, ~12us). Too-fat slices gate
  compute on their full completion instead; ~128KB slices interleaved in
  consumption order keep both issue (~0.66us each) and transfer
  (~200GB/s aggregate) ahead of the ~0.9us/batch pipeline.
- Tail: mask G with a transposed per-anchor onehot ([72+48, 256] bf16,
  both written into one SBUF tile at partition offset 0) and ship it in a
  single DMA. The group sums, per-anchor combine, and exact diagonal
  correction are O(n) with host-known coefficients and run on the host;
  the device tail is just mask-mul -> mask-mul -> DMA.
"""

import numpy as np
import ml_dtypes

N, D, KK, C = 2048, 128, 4, 24
NCORES = 8
JPC = N // NCORES  # 256 anchor rows per core
NBLK = JPC // 128  # 2 blocks of 128 rows
NCH = N // 128     # 16 l-chunks
EPS = 1e-6
OMEGA = 0.1
S_CENTER = 3.47
EPS2 = float(np.float32(EPS * np.exp(-S_CENTER)))

_cache: dict = {}


def _build(repeats: int, dma_scheme: int = 2, superbatch: bool = False, par_out: bool = True, blobs_gpsimd: bool = False):
    import concourse.bacc as bacc
    import concourse.mybir as mybir
    import concourse.tile as tile

    f32 = mybir.dt.float32
    bf16 = mybir.dt.bfloat16
    fp8 = mybir.dt.float8e4
    AL = mybir.AluOpType
    AF = mybir.ActivationFunctionType
    AX = mybir.AxisListType
    DR = mybir.MatmulPerfMode.DoubleRow

    nc = bacc.Bacc("TRN2", target_bir_lowering=False, debug=False)
    xt_d = nc.dram_tensor("xt", [D, N], bf16, kind="ExternalInput")
    # per chunk-pair k: [2,128] zero-padded g1 block, then [2,64] g2 block
    # (DoubleRow needs out partitions of 64/128; pad cols land in unread
    # PSUM rows)
    wt_d = nc.dram_tensor("wt", [128, NCH // 2, 384], fp8, kind="ExternalInput")
    blobA_d = nc.dram_tensor("blobA", [72, JPC], bf16, kind="ExternalInput")
    blobB_d = nc.dram_tensor("blobB", [48, JPC], bf16, kind="ExternalInput")
    ms_d = nc.dram_tensor("ms", [72, 2 * JPC], bf16, kind="ExternalOutput")

    with tile.TileContext(nc) as tc:
        with (
            tc.tile_pool(name="sb", bufs=4) as work,
            tc.tile_pool(name="ps", bufs=2, space="PSUM") as psim,
        ):
            const = work
            ps = psim
            def body():
                xt = const.tile([D, N], bf16, tag="xt")
                wt = const.tile([128, NCH // 2, 384], fp8, tag="wt")
                # few fat DMAs (descriptor issue is the constraint), ordered
                # so the first slices land just ahead of PE consumption
                if dma_scheme == 0:
                    nc.sync.dma_start(xt[:, 0:256], xt_d[:, 0:256])
                    nc.sync.dma_start(wt[:, 0:1, :], wt_d[:, 0:1, :])
                    nc.sync.dma_start(xt[:, 256:1024], xt_d[:, 256:1024])
                    nc.sync.dma_start(wt[:, 1:8, :], wt_d[:, 1:8, :])
                    nc.sync.dma_start(xt[:, 1024:2048], xt_d[:, 1024:2048])
                elif dma_scheme == 2:
                    # stream per 2-batch granularity: transfer (~1.1us) and
                    # issue (~1.3us) both stay ahead of ~1.8us consumption
                    for kk in range(4):
                        nc.sync.dma_start(xt[:, kk * 512:(kk + 1) * 512],
                                          xt_d[:, kk * 512:(kk + 1) * 512])
                        nc.sync.dma_start(wt[:, 2 * kk:2 * kk + 2, :],
                                          wt_d[:, 2 * kk:2 * kk + 2, :])
                elif dma_scheme == 4:
                    # small first slice (batch 0 starts earliest), still 8
                    # DMAs total
                    nc.sync.dma_start(xt[:, 0:256], xt_d[:, 0:256])
                    nc.sync.dma_start(wt[:, 0:1, :], wt_d[:, 0:1, :])
                    nc.sync.dma_start(xt[:, 256:768], xt_d[:, 256:768])
                    nc.sync.dma_start(wt[:, 1:3, :], wt_d[:, 1:3, :])
                    nc.sync.dma_start(xt[:, 768:1536], xt_d[:, 768:1536])
                    nc.sync.dma_start(wt[:, 3:6, :], wt_d[:, 3:6, :])
                    nc.sync.dma_start(xt[:, 1536:2048], xt_d[:, 1536:2048])
                    nc.sync.dma_start(wt[:, 6:8, :], wt_d[:, 6:8, :])
                elif dma_scheme == 3:
                    # like 2, but a small first slice so batch 0 starts
                    # ~0.5us earlier
                    nc.sync.dma_start(xt[:, 0:256], xt_d[:, 0:256])
                    nc.sync.dma_start(wt[:, 0:1, :], wt_d[:, 0:1, :])
                    nc.sync.dma_start(xt[:, 256:512], xt_d[:, 256:512])
                    nc.sync.dma_start(wt[:, 1:2, :], wt_d[:, 1:2, :])
                    nc.sync.dma_start(xt[:, 512:1024], xt_d[:, 512:1024])
                    nc.sync.dma_start(wt[:, 2:4, :], wt_d[:, 2:4, :])
                    nc.sync.dma_start(xt[:, 1024:1536], xt_d[:, 1024:1536])
                    nc.sync.dma_start(wt[:, 4:6, :], wt_d[:, 4:6, :])
                    nc.sync.dma_start(xt[:, 1536:2048], xt_d[:, 1536:2048])
                    nc.sync.dma_start(wt[:, 6:8, :], wt_d[:, 6:8, :])
                else:
                    nc.sync.dma_start(xt[:, 0:256], xt_d[:, 0:256])
                    nc.sync.dma_start(wt[:, 0:1, :], wt_d[:, 0:1, :])
                    nc.sync.dma_start(xt[:, 256:512], xt_d[:, 256:512])
                    nc.sync.dma_start(wt[:, 1:3, :], wt_d[:, 1:3, :])
                    nc.sync.dma_start(xt[:, 512:1024], xt_d[:, 512:1024])
                    nc.sync.dma_start(wt[:, 3:8, :], wt_d[:, 3:8, :])
                    nc.sync.dma_start(xt[:, 1024:2048], xt_d[:, 1024:2048])
                blobA = const.tile([72, JPC], bf16, tag="blobA")
                blobB = const.tile([48, JPC], bf16, tag="blobB")
                eng_b = nc.gpsimd if blobs_gpsimd else nc.sync
                eng_b.dma_start(blobA[:], blobA_d[:])
                eng_b.dma_start(blobB[:], blobB_d[:])

                epsb = const.tile([128, 1], f32, tag="epsb")
                nc.vector.memset(epsb[:], EPS2)
                ms = const.tile([72, 2 * JPC], bf16, tag="ms")
                # rows 48-71 of the m2 half are never written; zero them so
                # the shipped tile is fully initialized
                nc.vector.memset(ms[:, JPC:2 * JPC], 0.0)

                xjt = xt[:, 0:JPC]
                g1t = ps.tile([128, JPC], f32, tag="g1t", name="g1t")
                g2t = ps.tile([64, JPC], f32, tag="g2t", name="g2t")

                # four l-chunks per superbatch: the sim region spans two
                # ADJACENT psum banks (bank addresses are contiguous), so one
                # Ln/square covers 1024 elements, amortizing the ~150ns
                # per-instruction cost on the saturated scalar engine
                CPB = 4 if superbatch else 2
                NSB = NCH // CPB
                for k in range(NSB):
                    sim = psim.tile([128, CPB * JPC], f32, tag="sim")
                    for h in range(CPB):
                        i = CPB * k + h
                        nc.tensor.matmul(sim[:, h * JPC:(h + 1) * JPC],
                                         xt[:, i * 128:(i + 1) * 128], xjt,
                                         start=True, stop=True)
                    a = work.tile([128, CPB * JPC], fp8, tag="a")
                    nc.scalar.activation(a[:], sim[:], AF.Ln, bias=epsb[:])
                    a2 = work.tile([128, CPB * JPC], fp8, tag="a2")
                    nc.vector.tensor_mul(a2[:], a[:], a[:])
                    # fp8 DoubleRow g-matmuls per chunk pair
                    for half in range(CPB // 2):
                        pr = CPB // 2 * k + half
                        asl = a[:, half * 2 * JPC:(half + 1) * 2 * JPC]
                        a2sl = a2[:, half * 2 * JPC:(half + 1) * 2 * JPC]
                        av = asl.rearrange("p (s j) -> p s j", s=2)
                        a2v = a2sl.rearrange("p (s j) -> p s j", s=2)
                        w1v = wt[:, pr, 0:256].rearrange("p (s c) -> p s c", s=2)
                        w2v = wt[:, pr, 256:384].rearrange("p (s c) -> p s c", s=2)
                        nc.tensor.matmul(g1t[:], w1v, av,
                                         start=(pr == 0), stop=(pr == NCH // 2 - 1),
                                         perf_mode=DR)
                        nc.tensor.matmul(g2t[:], w2v, a2v,
                                         start=(pr == 0), stop=(pr == NCH // 2 - 1),
                                         perf_mode=DR)

                # select class column t_j: mask with the transposed onehot
                # and ship both masked products in one DMA; the 24-row group
                # sums are O(n) and run on the host with exact coefficients.
                nc.vector.tensor_mul(ms[:, 0:JPC], g1t[0:72, :], blobA[:])
                if par_out:
                    nc.sync.dma_start(ms_d[:, 0:JPC], ms[:, 0:JPC])
                    nc.vector.tensor_mul(ms[0:48, JPC:2 * JPC], g2t[0:48, :],
                                         blobB[:])
                    nc.scalar.dma_start(ms_d[0:48, JPC:2 * JPC],
                                        ms[0:48, JPC:2 * JPC])
                else:
                    nc.vector.tensor_mul(ms[0:48, JPC:2 * JPC], g2t[0:48, :],
                                         blobB[:])
                    nc.sync.dma_start(ms_d[:], ms[:])

            if repeats == 1:
                body()
            else:
                with tc.For_i(0, repeats, 1):
                    body()

    nc.compile()
    return nc


def _prep_inputs(inputs: np.ndarray, labels: np.ndarray):
    X = np.asarray(inputs, dtype=np.float32)
    lab = np.asarray(labels).astype(np.int64)
    t = lab[:, 0]
    scale = np.float32(np.exp(-S_CENTER / 2))
    XTb = np.ascontiguousarray((X * scale).astype(ml_dtypes.bfloat16).T)  # [128, N]

    E = (lab[:, :, None] == np.arange(C)[None, None, :]).astype(np.float32)  # [N,4,C]
    Wpos = E[:, 0]
    W0 = 1.0 - E[:, 3]
    W1 = E[:, 3] * (1.0 - E[:, 2])
    W2 = E[:, 2] * (1.0 - E[:, 1])
    W3 = E[:, 1] * (1.0 - E[:, 0])
    cm = np.array(
        [0.1 * (np.log(OMEGA + EPS) - np.log(OMEGA ** (KK - m + 1) + EPS)) for m in range(KK)],
        dtype=np.float32,
    )
    Wsum = W0 + W1 + W2 + W3
    # integer table: Wc = kappa * Wci up to ~1% of the tiny eps-correction in
    # cm (net ~1e-4 on the loss); small ints are exact in fp8e4
    Wci = 4.0 * W0 + 3.0 * W1 + 2.0 * W2 + W3
    kappa = float(cm[3])
    W120 = np.concatenate([Wpos, Wsum, Wci, Wpos, Wsum], axis=1)  # [N, 120]

    colsum = np.stack([W.sum(axis=0) for W in (W0, W1, W2, W3)])  # [4, C]
    cnt0 = Wpos.sum(axis=0)  # [C]
    NnS_c = colsum.sum(axis=0)
    NnC_c = (cm[:, None] * colsum).sum(axis=0)
    NnC2_c = ((cm ** 2)[:, None] * colsum).sum(axis=0)
    aux = np.stack(
        [NnS_c[t], NnC_c[t], NnC2_c[t], cnt0[t] - 1.0], axis=1
    ).astype(np.float32)  # [N, 4]

    in_maps = []
    for core in range(NCORES):
        j0 = core * JPC
        xt_core = np.concatenate([XTb[:, j0:], XTb[:, :j0]], axis=1)
        Wr = np.concatenate([W120[j0:], W120[:j0]], axis=0)
        R = Wr.reshape(NCH // 2, 2, 128, 120)
        R72 = np.zeros((NCH // 2, 2, 128, 128), dtype=np.float32)
        R72[:, :, :, 0:72] = R[:, :, :, 0:72]
        R48 = np.zeros((NCH // 2, 2, 128, 64), dtype=np.float32)
        R48[:, :, :, 0:48] = R[:, :, :, 72:120]
        blk = np.concatenate([
            R72.transpose(0, 2, 1, 3).reshape(NCH // 2, 128, 256),
            R48.transpose(0, 2, 1, 3).reshape(NCH // 2, 128, 128),
        ], axis=2)  # [8, 128, 384]: per pair [2,128] g1 block then [2,64] g2
        wt_core = np.ascontiguousarray(
            blk.transpose(1, 0, 2)
        ).astype(ml_dtypes.float8_e4m3)
        tcore = t[j0:j0 + JPC]
        oh24 = (np.arange(C)[:, None] == tcore[None, :]).astype(np.float32)  # [24,256]
        blobA = np.tile(oh24, (3, 1))
        blobB = np.tile(oh24, (2, 1))
        in_maps.append({
            "xt": xt_core,
            "wt": wt_core,
            "blobA": blobA.astype(ml_dtypes.bfloat16),
            "blobB": blobB.astype(ml_dtypes.bfloat16),
        })
    return in_maps


def _get_nc(repeats: int = 1, dma_scheme: int = 2, superbatch: bool = False,
            par_out: bool = True, blobs_gpsimd: bool = False):
    key = ("nc", repeats, dma_scheme, superbatch, par_out, blobs_gpsimd)
    if key not in _cache:
        _cache[key] = _build(repeats, dma_scheme, superbatch, par_out,
                             blobs_gpsimd)
    return _cache[key]


def _host_ctx(inputs, labels):
    """Per-anchor combine coefficients + exact diagonal correction (host side).

    The device ships qs rows [s1g, t1s, 2*T1C, s2g, t2s]; the remaining
    per-anchor combine is O(n) with label-only coefficients.
    """
    X = np.asarray(inputs, dtype=np.float32)
    lab = np.asarray(labels).astype(np.int64)
    t = lab[:, 0]
    scale = np.float32(np.exp(-S_CENTER / 2))
    Xb = (X * scale).astype(ml_dtypes.bfloat16).astype(np.float32)
    dA = np.log((Xb * Xb).sum(axis=1) + np.float32(EPS2))  # [N]
    E = (lab[:, :, None] == np.arange(C)[None, None, :]).astype(np.float32)
    Wpos = E[:, 0]
    W0 = 1.0 - E[:, 3]
    W1 = E[:, 3] * (1.0 - E[:, 2])
    W2 = E[:, 2] * (1.0 - E[:, 1])
    W3 = E[:, 1] * (1.0 - E[:, 0])
    cm = np.array(
        [0.1 * (np.log(OMEGA + EPS) - np.log(OMEGA ** (KK - m + 1) + EPS)) for m in range(KK)],
        dtype=np.float32,
    )
    colsum = np.stack([W.sum(axis=0) for W in (W0, W1, W2, W3)])
    cnt0 = Wpos.sum(axis=0)
    NnS = colsum.sum(axis=0)[t]
    NnC = (cm[:, None] * colsum).sum(axis=0)[t]
    NnC2 = ((cm ** 2)[:, None] * colsum).sum(axis=0)[t]
    Pn = cnt0[t] - 1.0
    kappa = float(cm[3])
    return dA, NnS, NnC, NnC2, Pn, kappa


def run_on_device(inputs, labels, repeats: int = 1):
    from concourse.bass_utils import run_bass_kernel_spmd

    nc = _get_nc(repeats)
    in_maps = _prep_inputs(inputs, labels)
    res = run_bass_kernel_spmd(nc, in_maps, list(range(NCORES)))
    dA, NnS, NnC, NnC2, Pn, kappa = _host_ctx(inputs, labels)
    dA2 = dA * dA
    partials = []
    for core in range(NCORES):
        j0 = core * JPC
        sl = slice(j0, j0 + JPC)
        M = np.asarray(res.results[core]["ms"], dtype=np.float32)
        s1g = M[0:24, 0:JPC].sum(axis=0)
        t1s = M[24:48, 0:JPC].sum(axis=0)
        t1c = M[48:72, 0:JPC].sum(axis=0)
        s2g = M[0:24, JPC:2 * JPC].sum(axis=0)
        t2s = M[24:48, JPC:2 * JPC].sum(axis=0)
        S1 = s1g - dA[sl]
        S2 = s2g - dA2[sl]
        L = (NnS[sl] * S2 - 2.0 * S1 * (NnC[sl] + t1s)
             + Pn[sl] * (NnC2[sl] + 2.0 * kappa * t1c + t2s))
        partials.append(np.float32(L.sum()))
    total = np.float32(np.sum(np.asarray(partials, dtype=np.float32)))
    return total, partials


def kernel(inputs, labels):
    total, _ = run_on_device(inputs, labels, repeats=1)
    return (total, 0, 0, 0)


# revision 65
# speedup vs baseline: 1.0119x; 1.0119x over previous
"""Bass/Trainium2 kernel for nn_LogRatio loss, data-parallel over anchor rows on 8 cores.

Math: loss = sum_{m,j,k,l} pos[j,k] * N_m[j,l] * (A[j,k] - A[j,l] - c_m)^2
with A = log(X @ X.T + eps). All masks depend on labels only through the
anchor's class t_j (24 classes), so every masked row-reduction becomes a
matmul of A against per-class 0/1 tables W[l, c] followed by a per-row
selection of column c = t_j.

Key layout choices vs the naive version:
- The loss only involves differences A[j,k] - A[j,l], so A may be shifted by
  any constant: fold exp(-s/2) into X on the host (s ~ mean log-sim). The
  centered A' in [-0.6, 0.4] makes bf16 rounding of the matmul operands
  harmless, enabling all-bf16 PE work (fp32 matmuls are 4x slower per column
  and need 4x-slower weight loads).
- W tables are the stationary matmul operand (72/48 columns), so the G sums
  accumulate transposed ([class, j]) in PSUM across the 16 l-chunks, with
  only 120 ldweights columns per chunk instead of 4 full 128x128 fp32 loads.
- Per-core inputs are rotated along l so each core's own anchor block is the
  first 256 columns of its X^T copy: the j-block ("xjt") needs no separate
  DMA and compute starts as soon as the first slice lands.
- Medium streamed DMAs: the naive version's DMA phase was sync-engine
  descriptor-issue bound (~2.6k descriptors, ~12us). Too-fat slices gate
  compute on their full completion instead; ~128KB slices interleaved in
  consumption order keep both issue (~0.66us each) and transfer
  (~200GB/s aggregate) ahead of the ~0.9us/batch pipeline.
- Tail: mask G with a transposed per-anchor onehot ([72+48, 256] bf16,
  both written into one SBUF tile at partition offset 0) and ship it in a
  single DMA. The group sums, per-anchor combine, and exact diagonal
  correction are O(n) with host-known coefficients and run on the host;
  the device tail is just mask-mul -> mask-mul -> DMA.
"""

import numpy as np
import ml_dtypes

N, D, KK, C = 2048, 128, 4, 24
NCORES = 8
JPC = N // NCORES  # 256 anchor rows per core
NBLK = JPC // 128  # 2 blocks of 128 rows
NCH = N // 128     # 16 l-chunks
EPS = 1e-6
OMEGA = 0.1
S_CENTER = 3.47
EPS2 = float(np.float32(EPS * np.exp(-S_CENTER)))

_cache: dict = {}


def _build(repeats: int, dma_scheme: int = 2, superbatch: bool = False, par_out: bool = True, blobs_gpsimd: bool = False):
    import concourse.bacc as bacc
    import concourse.mybir as mybir
    import concourse.tile as tile

    f32 = mybir.dt.float32
    bf16 = mybir.dt.bfloat16
    fp8 = mybir.dt.float8e4
    AL = mybir.AluOpType
    AF = mybir.ActivationFunctionType
    AX = mybir.AxisListType
    DR = mybir.MatmulPerfMode.DoubleRow

    nc = bacc.Bacc("TRN2", target_bir_lowering=False, debug=False)
    xt_d = nc.dram_tensor("xt", [D, N], bf16, kind="ExternalInput")
    # per chunk-pair k: [2,128] zero-padded g1 block, then [2,64] g2 block
    # (DoubleRow needs out partitions of 64/128; pad cols land in unread
    # PSUM rows)
    wt_d = nc.dram_tensor("wt", [128, NCH // 2, 384], fp8, kind="ExternalInput")
    blobA_d = nc.dram_tensor("blobA", [72, JPC], bf16, kind="ExternalInput")
    blobB_d = nc.dram_tensor("blobB", [48, JPC], bf16, kind="ExternalInput")
    ms_d = nc.dram_tensor("ms", [72, 2 * JPC], bf16, kind="ExternalOutput")

    with tile.TileContext(nc) as tc:
        with (
            tc.tile_pool(name="sb", bufs=4) as work,
            tc.tile_pool(name="ps", bufs=2, space="PSUM") as psim,
        ):
            const = work
            ps = psim
            def body():
                xt = const.tile([D, N], bf16, tag="xt")
                wt = const.tile([128, NCH // 2, 384], fp8, tag="wt")
                # few fat DMAs (descriptor issue is the constraint), ordered
                # so the first slices land just ahead of PE consumption
                if dma_scheme == 0:
                    nc.sync.dma_start(xt[:, 0:256], xt_d[:, 0:256])
                    nc.sync.dma_start(wt[:, 0:1, :], wt_d[:, 0:1, :])
                    nc.sync.dma_start(xt[:, 256:1024], xt_d[:, 256:1024])
                    nc.sync.dma_start(wt[:, 1:8, :], wt_d[:, 1:8, :])
                    nc.sync.dma_start(xt[:, 1024:2048], xt_d[:, 1024:2048])
                elif dma_scheme == 2:
                    # stream per 2-batch granularity: transfer (~1.1us) and
                    # issue (~1.3us) both stay ahead of ~1.8us consumption
                    for kk in range(4):
                        nc.sync.dma_start(xt[:, kk * 512:(kk + 1) * 512],
                                          xt_d[:, kk * 512:(kk + 1) * 512])
                        nc.sync.dma_start(wt[:, 2 * kk:2 * kk + 2, :],
                                          wt_d[:, 2 * kk:2 * kk + 2, :])
                elif dma_scheme == 4:
                    # small first slice (batch 0 starts earliest), still 8
                    # DMAs total
                    nc.sync.dma_start(xt[:, 0:256], xt_d[:, 0:256])
                    nc.sync.dma_start(wt[:, 0:1, :], wt_d[:, 0:1, :])
                    nc.sync.dma_start(xt[:, 256:768], xt_d[:, 256:768])
                    nc.sync.dma_start(wt[:, 1:3, :], wt_d[:, 1:3, :])
                    nc.sync.dma_start(xt[:, 768:1536], xt_d[:, 768:1536])
                    nc.sync.dma_start(wt[:, 3:6, :], wt_d[:, 3:6, :])
                    nc.sync.dma_start(xt[:, 1536:2048], xt_d[:, 1536:2048])
                    nc.sync.dma_start(wt[:, 6:8, :], wt_d[:, 6:8, :])
                elif dma_scheme == 3:
                    # like 2, but a small first slice so batch 0 starts
                    # ~0.5us earlier
                    nc.sync.dma_start(xt[:, 0:256], xt_d[:, 0:256])
                    nc.sync.dma_start(wt[:, 0:1, :], wt_d[:, 0:1, :])
                    nc.sync.dma_start(xt[:, 256:512], xt_d[:, 256:512])
                    nc.sync.dma_start(wt[:, 1:2, :], wt_d[:, 1:2, :])
                    nc.sync.dma_start(xt[:, 512:1024], xt_d[:, 512:1024])
                    nc.sync.dma_start(wt[:, 2:4, :], wt_d[:, 2:4, :])
                    nc.sync.dma_start(xt[:, 1024:1536], xt_d[:, 1024:1536])
                    nc.sync.dma_start(wt[:, 4:6, :], wt_d[:, 4:6, :])
                    nc.sync.dma_start(xt[:, 1536:2048], xt_d[:, 1536:2048])
                    nc.sync.dma_start(wt[:, 6:8, :], wt_d[:, 6:8, :])
                else:
                    nc.sync.dma_start(xt[:, 0:256], xt_d[:, 0:256])
                    nc.sync.dma_start(wt[:, 0:1, :], wt_d[:, 0:1, :])
                    nc.sync.dma_start(xt[:, 256:512], xt_d[:, 256:512])
                    nc.sync.dma_start(wt[:, 1:3, :], wt_d[:, 1:3, :])
                    nc.sync.dma_start(xt[:, 512:1024], xt_d[:, 512:1024])
                    nc.sync.dma_start(wt[:, 3:8, :], wt_d[:, 3:8, :])
                    nc.sync.dma_start(xt[:, 1024:2048], xt_d[:, 1024:2048])
                blobA = const.tile([72, JPC], bf16, tag="blobA")
                blobB = const.tile([48, JPC], bf16, tag="blobB")
                eng_b = nc.gpsimd if blobs_gpsimd else nc.sync
                eng_b.dma_start(blobA[:], blobA_d[:])
                eng_b.dma_start(blobB[:], blobB_d[:])

                epsb = const.tile([128, 1], f32, tag="epsb")
                nc.vector.memset(epsb[:], EPS2)
                ms = const.tile([72, 2 * JPC], bf16, tag="ms")

                xjt = xt[:, 0:JPC]
                g1t = ps.tile([128, JPC], f32, tag="g1t", name="g1t")
                g2t = ps.tile([64, JPC], f32, tag="g2t", name="g2t")

                # four l-chunks per superbatch: the sim region spans two
                # ADJACENT psum banks (bank addresses are contiguous), so one
                # Ln/square covers 1024 elements, amortizing the ~150ns
                # per-instruction cost on the saturated scalar engine
                CPB = 4 if superbatch else 2
                NSB = NCH // CPB
                for k in range(NSB):
                    sim = psim.tile([128, CPB * JPC], f32, tag="sim")
                    for h in range(CPB):
                        i = CPB * k + h
                        nc.tensor.matmul(sim[:, h * JPC:(h + 1) * JPC],
                                         xt[:, i * 128:(i + 1) * 128], xjt,
                                         start=True, stop=True)
                    a = work.tile([128, CPB * JPC], fp8, tag="a")
                    nc.scalar.activation(a[:], sim[:], AF.Ln, bias=epsb[:])
                    a2 = work.tile([128, CPB * JPC], fp8, tag="a2")
                    nc.vector.tensor_mul(a2[:], a[:], a[:])
                    # fp8 DoubleRow g-matmuls per chunk pair
                    for half in range(CPB // 2):
                        pr = CPB // 2 * k + half
                        asl = a[:, half * 2 * JPC:(half + 1) * 2 * JPC]
                        a2sl = a2[:, half * 2 * JPC:(half + 1) * 2 * JPC]
                        av = asl.rearrange("p (s j) -> p s j", s=2)
                        a2v = a2sl.rearrange("p (s j) -> p s j", s=2)
                        w1v = wt[:, pr, 0:256].rearrange("p (s c) -> p s c", s=2)
                        w2v = wt[:, pr, 256:384].rearrange("p (s c) -> p s c", s=2)
                        nc.tensor.matmul(g1t[:], w1v, av,
                                         start=(pr == 0), stop=(pr == NCH // 2 - 1),
                                         perf_mode=DR)
                        nc.tensor.matmul(g2t[:], w2v, a2v,
                                         start=(pr == 0), stop=(pr == NCH // 2 - 1),
                                         perf_mode=DR)

                # select class column t_j: mask with the transposed onehot
                # and ship both masked products in one DMA; the 24-row group
                # sums are O(n) and run on the host with exact coefficients.
                nc.vector.tensor_mul(ms[:, 0:JPC], g1t[0:72, :], blobA[:])
                if par_out:
                    # slow issuer (scalar, ~1us) takes the EARLY half whose
                    # latency hides behind the m2 mul; sync (0.8us) closes
                    nc.scalar.dma_start(ms_d[:, 0:JPC], ms[:, 0:JPC])
                    nc.vector.tensor_mul(ms[0:48, JPC:2 * JPC], g2t[0:48, :],
                                         blobB[:])
                    nc.sync.dma_start(ms_d[0:48, JPC:2 * JPC],
                                      ms[0:48, JPC:2 * JPC])
                else:
                    nc.vector.tensor_mul(ms[0:48, JPC:2 * JPC], g2t[0:48, :],
                                         blobB[:])
                    nc.sync.dma_start(ms_d[:], ms[:])

            if repeats == 1:
                body()
            else:
                with tc.For_i(0, repeats, 1):
                    body()

    nc.compile()
    return nc


def _prep_inputs(inputs: np.ndarray, labels: np.ndarray):
    X = np.asarray(inputs, dtype=np.float32)
    lab = np.asarray(labels).astype(np.int64)
    t = lab[:, 0]
    scale = np.float32(np.exp(-S_CENTER / 2))
    XTb = np.ascontiguousarray((X * scale).astype(ml_dtypes.bfloat16).T)  # [128, N]

    E = (lab[:, :, None] == np.arange(C)[None, None, :]).astype(np.float32)  # [N,4,C]
    Wpos = E[:, 0]
    W0 = 1.0 - E[:, 3]
    W1 = E[:, 3] * (1.0 - E[:, 2])
    W2 = E[:, 2] * (1.0 - E[:, 1])
    W3 = E[:, 1] * (1.0 - E[:, 0])
    cm = np.array(
        [0.1 * (np.log(OMEGA + EPS) - np.log(OMEGA ** (KK - m + 1) + EPS)) for m in range(KK)],
        dtype=np.float32,
    )
    Wsum = W0 + W1 + W2 + W3
    # integer table: Wc = kappa * Wci up to ~1% of the tiny eps-correction in
    # cm (net ~1e-4 on the loss); small ints are exact in fp8e4
    Wci = 4.0 * W0 + 3.0 * W1 + 2.0 * W2 + W3
    kappa = float(cm[3])
    W120 = np.concatenate([Wpos, Wsum, Wci, Wpos, Wsum], axis=1)  # [N, 120]

    colsum = np.stack([W.sum(axis=0) for W in (W0, W1, W2, W3)])  # [4, C]
    cnt0 = Wpos.sum(axis=0)  # [C]
    NnS_c = colsum.sum(axis=0)
    NnC_c = (cm[:, None] * colsum).sum(axis=0)
    NnC2_c = ((cm ** 2)[:, None] * colsum).sum(axis=0)
    aux = np.stack(
        [NnS_c[t], NnC_c[t], NnC2_c[t], cnt0[t] - 1.0], axis=1
    ).astype(np.float32)  # [N, 4]

    in_maps = []
    for core in range(NCORES):
        j0 = core * JPC
        xt_core = np.concatenate([XTb[:, j0:], XTb[:, :j0]], axis=1)
        Wr = np.concatenate([W120[j0:], W120[:j0]], axis=0)
        R = Wr.reshape(NCH // 2, 2, 128, 120)
        R72 = np.zeros((NCH // 2, 2, 128, 128), dtype=np.float32)
        R72[:, :, :, 0:72] = R[:, :, :, 0:72]
        R48 = np.zeros((NCH // 2, 2, 128, 64), dtype=np.float32)
        R48[:, :, :, 0:48] = R[:, :, :, 72:120]
        blk = np.concatenate([
            R72.transpose(0, 2, 1, 3).reshape(NCH // 2, 128, 256),
            R48.transpose(0, 2, 1, 3).reshape(NCH // 2, 128, 128),
        ], axis=2)  # [8, 128, 384]: per pair [2,128] g1 block then [2,64] g2
        wt_core = np.ascontiguousarray(
            blk.transpose(1, 0, 2)
        ).astype(ml_dtypes.float8_e4m3)
        tcore = t[j0:j0 + JPC]
        oh24 = (np.arange(C)[:, None] == tcore[None, :]).astype(np.float32)  # [24,256]
        blobA = np.tile(oh24, (3, 1))
        blobB = np.tile(oh24, (2, 1))
        in_maps.append({
            "xt": xt_core,
            "wt": wt_core,
            "blobA": blobA.astype(ml_dtypes.bfloat16),
            "blobB": blobB.astype(ml_dtypes.bfloat16),
        })
    return in_maps


def _get_nc(repeats: int = 1, dma_scheme: int = 2, superbatch: bool = False,
            par_out: bool = True, blobs_gpsimd: bool = False):
    key = ("nc", repeats, dma_scheme, superbatch, par_out, blobs_gpsimd)
    if key not in _cache:
        _cache[key] = _build(repeats, dma_scheme, superbatch, par_out,
                             blobs_gpsimd)
    return _cache[key]


def _host_ctx(inputs, labels):
    """Per-anchor combine coefficients + exact diagonal correction (host side).

    The device ships qs rows [s1g, t1s, 2*T1C, s2g, t2s]; the remaining
    per-anchor combine is O(n) with label-only coefficients.
    """
    X = np.asarray(inputs, dtype=np.float32)
    lab = np.asarray(labels).astype(np.int64)
    t = lab[:, 0]
    scale = np.float32(np.exp(-S_CENTER / 2))
    Xb = (X * scale).astype(ml_dtypes.bfloat16).astype(np.float32)
    dA = np.log((Xb * Xb).sum(axis=1) + np.float32(EPS2))  # [N]
    E = (lab[:, :, None] == np.arange(C)[None, None, :]).astype(np.float32)
    Wpos = E[:, 0]
    W0 = 1.0 - E[:, 3]
    W1 = E[:, 3] * (1.0 - E[:, 2])
    W2 = E[:, 2] * (1.0 - E[:, 1])
    W3 = E[:, 1] * (1.0 - E[:, 0])
    cm = np.array(
        [0.1 * (np.log(OMEGA + EPS) - np.log(OMEGA ** (KK - m + 1) + EPS)) for m in range(KK)],
        dtype=np.float32,
    )
    colsum = np.stack([W.sum(axis=0) for W in (W0, W1, W2, W3)])
    cnt0 = Wpos.sum(axis=0)
    NnS = colsum.sum(axis=0)[t]
    NnC = (cm[:, None] * colsum).sum(axis=0)[t]
    NnC2 = ((cm ** 2)[:, None] * colsum).sum(axis=0)[t]
    Pn = cnt0[t] - 1.0
    kappa = float(cm[3])
    return dA, NnS, NnC, NnC2, Pn, kappa


def run_on_device(inputs, labels, repeats: int = 1):
    from concourse.bass_utils import run_bass_kernel_spmd

    nc = _get_nc(repeats)
    in_maps = _prep_inputs(inputs, labels)
    res = run_bass_kernel_spmd(nc, in_maps, list(range(NCORES)))
    dA, NnS, NnC, NnC2, Pn, kappa = _host_ctx(inputs, labels)
    dA2 = dA * dA
    partials = []
    for core in range(NCORES):
        j0 = core * JPC
        sl = slice(j0, j0 + JPC)
        M = np.asarray(res.results[core]["ms"], dtype=np.float32)
        s1g = M[0:24, 0:JPC].sum(axis=0)
        t1s = M[24:48, 0:JPC].sum(axis=0)
        t1c = M[48:72, 0:JPC].sum(axis=0)
        s2g = M[0:24, JPC:2 * JPC].sum(axis=0)
        t2s = M[24:48, JPC:2 * JPC].sum(axis=0)
        S1 = s1g - dA[sl]
        S2 = s2g - dA2[sl]
        L = (NnS[sl] * S2 - 2.0 * S1 * (NnC[sl] + t1s)
             + Pn[sl] * (NnC2[sl] + 2.0 * kappa * t1c + t2s))
        partials.append(np.float32(L.sum()))
    total = np.float32(np.sum(np.asarray(partials, dtype=np.float32)))
    return total, partials


def kernel(inputs, labels):
    total, _ = run_on_device(inputs, labels, repeats=1)
    return (total, 0, 0, 0)
